# revision 20
# baseline (speedup 1.0000x reference)
"""Trainium2 Bass kernel for LowRankRayTracer.

csi[f] = (delta_t/D) * v_f^T M v_f,  M = conj(rad)^T conj(att)  (R=32, complex)
contracted over N = D*K = 524288 rows.

Design (8 cores, ray-sharded, all-fp16 data path):
  - Host converts each core's ray shard to PLANAR fp16 rows u = [Re|Im] (64
    wide), 2-packed per partition: tile (128, 4096) x 8, so each 128-col slice
    is one lhsT/rhs pair contracting 256 rows.  fp16 halves HBM traffic vs the
    exact hi/lo fp32 split; quantization error ~6e-4 << 2e-2 gate.
  - 256 matmuls accumulate S (128x128 quadrants) in PSUM; split A (tiles 0-6)
    / B (tile 7) so the A-part of the S->W_real epilogue hides under tile-7's
    DMA.  W_real = [[Q,P],[P,-Q]] (Q=S_rr-S_ii, P=S_ri+S_ir, scale folded into
    selector consts) is built by 16 small selector matmuls accumulating into
    one PSUM tile, cast twice into whh = [W|W] (64x128 fp16): because
    T_imag = rot(T_real), one T = whh^T g matmul per chunk covers re+im.
  - Warm-keeper dummy matmuls fill PE idle during the DMA-bound stretch so
    the HAM clock gate stays at 8/8 (2.4 GHz) through the tail.
  - Phase 3 in 8 super-chunks of 1024 subcarriers: 2 T matmuls, one 1024-wide
    e = g_stack .* T (g_stack rows: vr,vi,vi,-vr; direct DVE-from-PSUM or
    ACT-cast + DVE fp16 2x, balanced), 2 c matmuls lagged two super-chunks so
    the in-order PE queue never blocks; all chunk sums land in two (16,512)
    PSUM tiles so the finalize is 2 copies + 2 output DMAs, first pair
    overlapped with the second half of the pipeline.
  - g_stack (2 MiB) is consumed chunk-by-chunk in phase 3 only, so its 8
    chunk DMAs are emitted AFTER the ray DMAs on the same FIFO rings: the
    last ray byte lands ~5 us earlier and each gs chunk still arrives ahead
    of its T matmul.
"""

import numpy as np

D, K, R = 4096, 128, 32
F = 8192
N_CORES = 8
DIR_PER_CORE = D // N_CORES              # 512
N_TILE = 8                               # DMA tiles per tensor per core
TILE_COLS = 4096                         # fp16 cols per partition per tile
N_SLICE = TILE_COLS // 128               # 32 matmul slices per tile
S_SCALE = (200.0 / K) / D                # delta_t / num_directions (exact)
FCHUNK = 512
N_SC = 8                                 # phase-3 super-chunks
SC = 2 * FCHUNK                          # 1024 subcarriers per super-chunk

_NC_CACHE = {}


def _build_sel():
    """(128, 384) fp16: C_IA|C_IB|C_XA|C_XB|-C_XA|-C_XB, scale folded in."""
    s = np.float16(S_SCALE)
    c = np.zeros((128, 384), np.float16)
    for p in range(64):
        c[p, p] = s            # C_IA
        c[64 + p, 64 + p] = s  # C_IB
    for i in range(32):
        c[i, 128 + 32 + i] = s       # C_XA
        c[32 + i, 128 + i] = -s
        c[64 + i, 192 + 32 + i] = s  # C_XB
        c[96 + i, 192 + i] = -s
    c[:, 256:320] = -c[:, 128:192]   # -C_XA
    c[:, 320:384] = -c[:, 192:256]   # -C_XB
    return c


def _build_csel():
    """(128, 256) fp16: chunk k block (128,16) sums rows 0:64 -> local col
    2*(k%8), rows 64:96 minus rows 96:128 -> 2*(k%8)+1 (im = vi.T_top -
    vr.T_bot; gs rows 96:128 hold +vr, sign lives here)."""
    c = np.zeros((128, 256), np.float16)
    for k in range(16):
        c[0:64, 16 * k + 2 * (k % 8)] = 1.0
        c[64:128, 16 * k + 2 * (k % 8) + 1] = 1.0
    return c


def build_nc():
    import concourse.bacc as bacc
    import concourse.mybir as mybir
    import concourse.tile as tile

    fp32 = mybir.dt.float32
    fp16 = mybir.dt.float16
    nc = bacc.Bacc(trn_type="TRN2", target_bir_lowering=False, debug=False)

    rad_d = nc.dram_tensor("rad", [N_TILE, 128, TILE_COLS], fp16,
                           kind="ExternalInput").ap()
    att_d = nc.dram_tensor("att", [N_TILE, 128, TILE_COLS], fp16,
                           kind="ExternalInput").ap()
    gs_d = nc.dram_tensor("gs", [128, F], fp16, kind="ExternalInput").ap()
    sel_d = nc.dram_tensor("sel", [128, 384], fp16, kind="ExternalInput").ap()
    csel_d = nc.dram_tensor("csel", [128, 256], fp16,
                            kind="ExternalInput").ap()
    out_d = nc.dram_tensor("csi", [32, FCHUNK], fp32,
                           kind="ExternalOutput").ap()

    with tile.TileContext(nc) as tc:
        with (
            tc.tile_pool(name="io", bufs=4) as io_pool,
            tc.tile_pool(name="small", bufs=1) as small,
            tc.tile_pool(name="epool", bufs=4) as epool,
        ):
            # constants + frequency stack on the gpsimd (SWDGE) queue so the
            # sync/scalar HWDGE rings stay dedicated to ray data
            sel_sb = small.tile([128, 384], fp16, tag="sel")
            nc.gpsimd.dma_start(sel_sb[:], sel_d[:])
            csel_sb = small.tile([128, 256], fp16, tag="csel")
            nc.gpsimd.dma_start(csel_sb[:], csel_d[:])
            # gs is only consumed by phase 3, chunk by chunk: stream it
            # AFTER the ray data on the same FIFO rings (emitted post-loop)
            gs_sb = small.tile([128, F], fp16, tag="gs")

            sfa = small.tile([128, 128], fp32, tag="sfa")
            s_sba = small.tile([128, 128], fp16, tag="s_sba")
            sfb = small.tile([128, 128], fp32, tag="sfb")
            s_sbb = small.tile([128, 128], fp16, tag="s_sbb")
            whh = small.tile([64, 128], fp16, tag="whh")
            IA = sel_sb[:, 0:64]
            IB = sel_sb[:, 64:128]
            XA = sel_sb[:, 128:192]
            XB = sel_sb[:, 192:256]
            XNA = sel_sb[:, 256:320]
            XNB = sel_sb[:, 320:384]
            # selector matmul plan: (lhsT, s_sb col, dest col block)
            WPLAN0 = ((IA, 0), (IB, 64), (XA, 32), (XB, 96))
            WPLAN1 = ((XNA, 0), (XNB, 64), (IA, 32), (IB, 96))

            DUMMIES = {3: 24, 4: 30, 5: 30, 6: 12}
            with tc.tile_pool(name="mpsum", bufs=1, space="PSUM") as mpsum:
                dummy_ps = mpsum.tile([128, 256], fp32, tag="dummy")
                wps = mpsum.tile([64, 64], fp32, tag="wps")

                def warm(n):
                    for _ in range(n):
                        nc.tensor.matmul(dummy_ps[:], lhsT=sel_sb[:, 0:128],
                                         rhs=sel_sb[:, 0:256], start=True,
                                         stop=True, skip_group_check=True)

                def wmm(s_sb, dst, plan, start, stop):
                    for j, (lh, rc) in enumerate(plan):
                        nc.tensor.matmul(wps[:, dst:dst + 32], lhsT=lh,
                                         rhs=s_sb[:, rc:rc + 32],
                                         start=(start and j == 0),
                                         stop=(stop and j == len(plan) - 1),
                                         skip_group_check=True)

                banksA = [mpsum.tile([128, 128], fp32, tag=f"sa{b}",
                                     name=f"sa{b}") for b in range(2)]
                idx = 0
                for i in range(N_TILE):
                    if i < N_TILE - 1:
                        rad_t = io_pool.tile([128, TILE_COLS], fp16,
                                             tag="rad")
                        att_t = io_pool.tile([128, TILE_COLS], fp16,
                                             tag="att")
                        nc.sync.dma_start(rad_t[:], rad_d[i, :, :])
                        nc.scalar.dma_start(att_t[:], att_d[i, :, :])
                    else:
                        # tile 7 in quarters so MMs chase the DMA tail
                        rad_t = io_pool.tile([128, TILE_COLS], fp16,
                                             tag="rad")
                        att_t = io_pool.tile([128, TILE_COLS], fp16,
                                             tag="att")
                        qc = TILE_COLS // 4
                        for q in range(4):
                            qs = slice(q * qc, (q + 1) * qc)
                            nc.sync.dma_start(rad_t[:, qs], rad_d[7, :, qs])
                            nc.scalar.dma_start(att_t[:, qs], att_d[7, :, qs])
                    for s in range(N_SLICE):
                        sl = slice(s * 128, (s + 1) * 128)
                        nc.tensor.matmul(banksA[idx % 2][:],
                                         lhsT=rad_t[:, sl], rhs=att_t[:, sl],
                                         start=(idx < 2),
                                         stop=(idx >= 8 * N_SLICE - 2))
                        idx += 1
                    warm(DUMMIES.get(i, 0))
                for g in range(N_SC):
                    gsl = slice(g * SC, (g + 1) * SC)
                    eng = nc.sync if g % 2 == 0 else nc.scalar
                    eng.dma_start(gs_sb[:, gsl], gs_d[:, gsl])
                warm(6)
                nc.vector.tensor_copy(sfa[:], banksA[0][:])
                nc.vector.tensor_add(s_sba[:], sfa[:], banksA[1][:])
                wmm(s_sba, 0, WPLAN0, start=True, stop=True)
                wmm(s_sba, 32, WPLAN1, start=True, stop=True)
                warm(4)
                nc.vector.tensor_copy(whh[:, 0:64], wps[:])
                nc.vector.tensor_copy(whh[:, 64:128], wps[:])

            # ---- phase 3: 8 super-chunks of 1024 subcarriers ----
            csiA = small.tile([16, FCHUNK], fp32, tag="csiA")
            csiB = small.tile([16, FCHUNK], fp32, tag="csiB")
            DIRECT = {2, 5, 7}
            with (
                tc.tile_pool(name="tpsum", bufs=3, space="PSUM") as tpsum,
                tc.tile_pool(name="cpsum", bufs=1, space="PSUM") as cpsum,
            ):
                c_accA = cpsum.tile([16, FCHUNK], fp32, tag="c_accA")
                c_accB = cpsum.tile([16, FCHUNK], fp32, tag="c_accB")
                e_tiles = {}

                def emit_c(sc):
                    c_acc = c_accA if sc < 4 else c_accB
                    for h in range(2):
                        k = 2 * sc + h
                        nc.tensor.matmul(
                            c_acc[:], lhsT=csel_sb[:, 16 * k:16 * k + 16],
                            rhs=e_tiles[sc][:, FCHUNK * h:FCHUNK * (h + 1)],
                            start=(k % 8 == 0), stop=(k % 8 == 7),
                            skip_group_check=True)

                for sc in range(N_SC):
                    fs = slice(sc * SC, (sc + 1) * SC)
                    t_ps = tpsum.tile([128, SC], fp32, tag="t", name=f"t{sc}")
                    for h in range(2):
                        hs = slice(FCHUNK * h, FCHUNK * (h + 1))
                        nc.tensor.matmul(t_ps[:, hs], lhsT=whh[:],
                                         rhs=gs_sb[0:64, sc * SC + FCHUNK * h:
                                                    sc * SC + FCHUNK * (h + 1)],
                                         start=True, stop=True,
                                         skip_group_check=True)
                    e_sb = epool.tile([128, SC], fp16, tag="e", name=f"e{sc}")
                    e_tiles[sc] = e_sb
                    if sc in DIRECT:
                        if sc == N_SC - 1:
                            # final chunk: halve the e-op so the end drain
                            # overlaps (first half starts after T7a alone)
                            for h in range(2):
                                hs = slice(FCHUNK * h, FCHUNK * (h + 1))
                                gh = slice(sc * SC + FCHUNK * h,
                                           sc * SC + FCHUNK * (h + 1))
                                nc.vector.tensor_mul(e_sb[:, hs],
                                                     gs_sb[:, gh],
                                                     t_ps[:, hs])
                        else:
                            nc.vector.tensor_mul(e_sb[:], gs_sb[:, fs],
                                                 t_ps[:])
                    else:
                        tc16 = epool.tile([128, SC], fp16, tag="tc",
                                          name=f"tc{sc}")
                        nc.scalar.copy(tc16[:], t_ps[:])
                        nc.vector.tensor_mul(e_sb[:], gs_sb[:, fs], tc16[:])
                    if sc >= 2:
                        emit_c(sc - 2)
                    if sc == 7:
                        # first-half finalize overlaps the last super-chunks
                        nc.scalar.copy(csiA[:], c_accA[:])
                        nc.sync.dma_start(out_d[0:16, :], csiA[:])
                emit_c(N_SC - 2)
                emit_c(N_SC - 1)
                nc.scalar.copy(csiB[:], c_accB[:])

            nc.sync.dma_start(out_d[16:32, :], csiB[:])

    nc.compile()
    return nc


def _pack_planar(arr, core):
    """Core's complex64 shard -> (N_TILE, 128, TILE_COLS) planar fp16."""
    sh = arr[core * DIR_PER_CORE:(core + 1) * DIR_PER_CORE]
    n = DIR_PER_CORE * K
    u = np.empty((n, 64), np.float16)
    u[:, :32] = sh.real.reshape(n, 32)
    u[:, 32:] = sh.imag.reshape(n, 32)
    t = u.reshape(N_TILE, N_SLICE, 2, 128, 64).transpose(0, 3, 1, 2, 4)
    return np.ascontiguousarray(t.reshape(N_TILE, 128, TILE_COLS))


def _build_gs(fbv):
    """(128, F) fp16 rows: vr(32), vi(32), vi(32), -vr(32)."""
    vr = np.ascontiguousarray(fbv.real.T)
    vi = np.ascontiguousarray(fbv.imag.T)
    return np.concatenate([vr, vi, vi, -vr], axis=0).astype(np.float16)


def _build_in_maps(attenuation_vectors, radiation_vectors,
                   frequency_basis_vectors):
    gs = _build_gs(frequency_basis_vectors)
    sel = _build_sel()
    csel = _build_csel()
    in_maps = []
    for c in range(N_CORES):
        in_maps.append({
            "rad": _pack_planar(radiation_vectors, c),
            "att": _pack_planar(attenuation_vectors, c),
            "gs": gs, "sel": sel, "csel": csel,
        })
    return in_maps


def kernel(attenuation_vectors, radiation_vectors, frequency_basis_vectors):
    from concourse.bass_utils import run_bass_kernel_spmd

    if "nc" not in _NC_CACHE:
        _NC_CACHE["nc"] = build_nc()
    nc = _NC_CACHE["nc"]

    in_maps = _build_in_maps(attenuation_vectors, radiation_vectors,
                             frequency_basis_vectors)
    res = run_bass_kernel_spmd(nc, in_maps, core_ids=list(range(N_CORES)))
    acc = np.zeros((32, FCHUNK), np.float64)
    for r in res.results:
        acc += r["csi"]
    re = acc[0::2].reshape(-1)
    im = acc[1::2].reshape(-1)
    return (re + 1j * im).astype(np.complex64)
